# revision 3
# baseline (speedup 1.0000x reference)
"""Trainium2 Bass kernel for nn_KANLayer (B=16384, D=1024, K=8).

Math: the per-feature basis chain collapses algebraically:
    nl[b,i] = sum_k (x[b,i]*W1[i,k] + b1[i,k]) * W2[i,k]
            = x[b,i] * a[i] + c[i],   a = sum_k W1*W2, c = sum_k b1*W2
so the whole layer is ONE dense matmul with a fused diagonal + bias:
    out = x @ (lin_W.T + diag(a)) + (lin_b + c)

Precision strategy (validated numerically on the fixed seed-0 inputs,
rel err 1.04e-2 vs the 2e-2 gate): split W_eff = W_off + diag(d).
  - x @ W_off runs on the PE in fp8 e4m3 with perf_mode=DoubleRow
    (2 fp8/cell, K=256 per matmul) — ~2x bf16 throughput.
  - the diagonal term d[i]*x[b,i] is large (|d|~1 vs |W_off|~1/32), so
    it is applied at eviction from a higher-precision x = x8 + xe8
    (fp8 value + fp8 error) on the vector engine.
  - output stored bf16 (rounding err ~0.4%, well inside the gate).

Layout: everything transposed — W is the PE-stationary operand, x^T the
moving one, so psum holds out^T[i, b]. The diagonal/bias then become
per-partition scalars (one ACT op + two DVE scalar_tensor_tensor ops
per tile). Host transposes the output back.

Sharding: data-parallel over batch across 8 NeuronCores (2048 rows
each); weights replicated. No collectives.
"""

from contextlib import ExitStack

import numpy as np
import ml_dtypes

import concourse.bass as bass
import concourse.tile as tile
from concourse import bacc, mybir
from concourse.bass_utils import run_bass_kernel_spmd

B, D = 16384, 1024
NCORES = 8
BS = B // NCORES   # 2048 batch rows per core
P = 128
TP = 4             # contraction k-pairs (each pair = 256 rows via DoubleRow)
IB = D // P        # 8 output-feature blocks of 128
NBC = BS // 512    # 4 batch chunks of 512

FP8 = mybir.dt.float8e4
FP8_NP = ml_dtypes.float8_e4m3fn  # matches TRN fp8e4 within +-240
BF16_NP = ml_dtypes.bfloat16

_CACHE = {}


def _build_nc():
    nc = bacc.Bacc("TRN2", target_bir_lowering=False, debug=False,
                   num_devices=NCORES)
    # x8/xe8: x^T per core, fp8 value + fp8 residual, tiled
    # [p, tp, 2, b] with j = (2*tp + sub)*128 + p the contraction index.
    x8 = nc.dram_tensor("x8", [P, TP, 2, BS], FP8, kind="ExternalInput").ap()
    xe8 = nc.dram_tensor("xe8", [P, TP, 2, BS], FP8,
                         kind="ExternalInput").ap()
    # W_off (diag zeroed), same contraction tiling: [p, tp, 2, i]
    w8 = nc.dram_tensor("w8", [P, TP, 2, D], FP8, kind="ExternalInput").ap()
    # diag + bias as per-partition columns: dv[p, ib] = d[ib*128+p]
    dv = nc.dram_tensor("dv", [P, IB], mybir.dt.float32,
                        kind="ExternalInput").ap()
    bv = nc.dram_tensor("bv", [P, IB], mybir.dt.float32,
                        kind="ExternalInput").ap()
    # out^T bf16: out[p, ib, b] = result[b, ib*128+p]
    out = nc.dram_tensor("out", [P, IB, BS], mybir.dt.bfloat16,
                         kind="ExternalOutput").ap()

    Act = mybir.ActivationFunctionType
    Alu = mybir.AluOpType
    DR = mybir.MatmulPerfMode.DoubleRow

    with tile.TileContext(nc) as tc, ExitStack() as ctx:
        cpool = ctx.enter_context(tc.tile_pool(name="cpool", bufs=1))
        opool = ctx.enter_context(tc.tile_pool(name="opool", bufs=3))
        tpool = ctx.enter_context(tc.tile_pool(name="tpool", bufs=8))
        ppool = ctx.enter_context(tc.tile_pool(name="ppool", bufs=8,
                                               space="PSUM"))

        # small per-partition vectors first (vector-engine HWDGE ring)
        dv_t = cpool.tile([P, IB], mybir.dt.float32, tag="dv", name="dv_t")
        bv_t = cpool.tile([P, IB], mybir.dt.float32, tag="bv", name="bv_t")
        nc.sync.dma_start(out=dv_t, in_=dv)
        nc.sync.dma_start(out=bv_t, in_=bv)

        # PE pre-warm with fp8 DoubleRow matmuls on a zero tile so the
        # HAM clock-gate ramps while the input DMAs run.
        warm = cpool.tile([P, 2, 512], FP8, tag="warm", name="warm")
        nc.vector.memset(warm, 0.0)
        warm_ps = ppool.tile([P, 512], mybir.dt.float32, tag="ps",
                             name="warm_ps")
        for i in range(6):
            nc.tensor.matmul(warm_ps, lhsT=warm[:, :, :P], rhs=warm,
                             start=(i == 0), stop=(i == 5), perf_mode=DR)

        # weights per k-pair on the scalar ring (first chunk gates PE)
        w_t = []
        for t in range(TP):
            wt = cpool.tile([P, 2, D], FP8, tag=f"w{t}", name=f"w_t{t}")
            nc.scalar.dma_start(out=wt, in_=w8[:, t])
            w_t.append(wt)

        # x^T per k-pair on the sync ring; first pair split by b-chunk so
        # the first matmul's input lands early.
        x_t = []
        for t in range(TP):
            xt = cpool.tile([P, 2, BS], FP8, tag=f"x{t}", name=f"x_t{t}")
            if t == 0:
                for b4 in range(NBC):
                    nc.sync.dma_start(out=xt[:, :, bass.ts(b4, 512)],
                                      in_=x8[:, t, :, bass.ts(b4, 512)])
            else:
                nc.sync.dma_start(out=xt, in_=x8[:, t])
            x_t.append(xt)

        # fp8 residual of x on the gpsimd ring (only needed at eviction)
        xe_t = []
        for t in range(TP):
            xet = cpool.tile([P, 2, BS], FP8, tag=f"xe{t}", name=f"xe_t{t}")
            nc.gpsimd.dma_start(out=xet, in_=xe8[:, t])
            xe_t.append(xet)

        for ib in range(IB):
            isl = bass.ts(ib, P)
            psums = [ppool.tile([P, 512], mybir.dt.float32, tag="ps",
                                name=f"ps{ib}_{bc}") for bc in range(NBC)]
            for t in range(TP):
                for bc in range(NBC):
                    nc.tensor.matmul(
                        psums[bc],
                        lhsT=w_t[t][:, :, isl],
                        rhs=x_t[t][:, :, bass.ts(bc, 512)],
                        start=(t == 0),
                        stop=(t == TP - 1),
                        perf_mode=DR,
                    )
            # eviction: out^T[i,b] = psum + d[i]*(x8+xe8)[i,b] + bias[i]
            t8, sub = divmod(ib, 2)
            o_t = opool.tile([P, BS], mybir.dt.bfloat16, tag="o",
                             name=f"o_t{ib}")
            for bc in range(NBC):
                bsl = bass.ts(bc, 512)
                tb = tpool.tile([P, 512], mybir.dt.float32, tag="tb",
                                name=f"tb{ib}_{bc}")
                nc.scalar.activation(tb, psums[bc], Act.Identity,
                                     bias=bv_t[:, ib:ib + 1], scale=1.0)
                s1 = tpool.tile([P, 512], mybir.dt.float32, tag="s1",
                                name=f"s1{ib}_{bc}")
                nc.vector.scalar_tensor_tensor(
                    s1, in0=xe_t[t8][:, sub, bsl], scalar=dv_t[:, ib:ib + 1],
                    in1=tb, op0=Alu.mult, op1=Alu.add)
                nc.vector.scalar_tensor_tensor(
                    o_t[:, bsl], in0=x_t[t8][:, sub, bsl],
                    scalar=dv_t[:, ib:ib + 1],
                    in1=s1, op0=Alu.mult, op1=Alu.add)
            nc.scalar.dma_start(out=out[:, ib], in_=o_t)

    nc.compile()
    return nc


def _get_nc():
    if "nc" not in _CACHE:
        _CACHE["nc"] = _build_nc()
    return _CACHE["nc"]


def _prep_inputs(x, lin_W, lin_b, W1, b1, W2):
    """Host-side prep: fold the basis chain, split W into off-diag + diag,
    quantize to fp8 (value + residual), and lay out transposed per core."""
    x = np.asarray(x, dtype=np.float32)
    lin_W = np.asarray(lin_W, dtype=np.float32)
    a = np.sum(np.asarray(W1, np.float32) * np.asarray(W2, np.float32),
               axis=1)
    c = np.sum(np.asarray(b1, np.float32) * np.asarray(W2, np.float32),
               axis=1)
    W_eff = np.ascontiguousarray(lin_W.T)
    idx = np.arange(D)
    W_eff[idx, idx] += a
    d = W_eff[idx, idx].copy()
    W_off = W_eff
    W_off[idx, idx] = 0.0
    bias = (np.asarray(lin_b, np.float32) + c).astype(np.float32)

    x8 = x.astype(FP8_NP)
    xe8 = (x - x8.astype(np.float32)).astype(FP8_NP)
    w8 = W_off.astype(FP8_NP)

    # w8 dram layout [p, tp, 2, i]: j = (2*tp+sub)*128 + p
    w8_dev = np.ascontiguousarray(
        w8.reshape(TP, 2, P, D).transpose(2, 0, 1, 3))
    dv_dev = np.ascontiguousarray(d.reshape(IB, P).T)
    bv_dev = np.ascontiguousarray(bias.reshape(IB, P).T)

    def xpose(arr):  # [NCORES*BS, D] fp8 -> per-core [p, tp, 2, b]
        t = arr.reshape(NCORES, BS, TP, 2, P)
        return np.ascontiguousarray(t.transpose(0, 4, 2, 3, 1))

    x8_dev = xpose(x8)
    xe8_dev = xpose(xe8)

    return [
        {"x8": x8_dev[i], "xe8": xe8_dev[i], "w8": w8_dev,
         "dv": dv_dev, "bv": bv_dev}
        for i in range(NCORES)
    ]


def kernel(x, lin_W, lin_b, W1, b1, W2):
    in_maps = _prep_inputs(x, lin_W, lin_b, W1, b1, W2)
    nc = _get_nc()
    res = run_bass_kernel_spmd(nc, in_maps, core_ids=list(range(NCORES)))
    # out^T [p, ib, b] per core -> [b_global, ib*128+p]
    o = np.stack([r["out"] for r in res.results])  # [cores, P, IB, BS] bf16
    o = o.astype(np.float32).transpose(0, 3, 2, 1).reshape(B, D)
    return np.ascontiguousarray(o)


# revision 9
# speedup vs baseline: 1.1894x; 1.1894x over previous
"""Trainium2 Bass kernel for nn_KANLayer (B=16384, D=1024, K=8).

Math: the per-feature basis chain collapses algebraically:
    nl[b,i] = sum_k (x[b,i]*W1[i,k] + b1[i,k]) * W2[i,k]
            = x[b,i] * a[i] + c[i],   a = sum_k W1*W2, c = sum_k b1*W2
so the whole layer is ONE dense matmul with a fused diagonal + bias:
    out = x @ (lin_W.T + diag(a)) + (lin_b + c)

Precision strategy (validated numerically on the fixed seed-0 inputs,
rel err 1.04e-2 vs the 2e-2 gate): split W_eff = W_off + diag(d).
  - x @ W_off runs on the PE in fp8 e4m3 with perf_mode=DoubleRow
    (2 fp8/cell, K=256 per matmul) — ~2x bf16 throughput.
  - the diagonal term d[i]*x[b,i] is large (|d|~1 vs |W_off|~1/32), so
    it is applied at eviction from a bf16 copy of x on the vector
    engine (one scalar_tensor_tensor op per tile).
  - output stored bf16 (rounding err ~0.4%, well inside the gate).

Layout: everything transposed — W is the PE-stationary operand, x^T the
moving one, so psum holds out^T[i, b]. The diagonal/bias then become
per-partition scalars (one ACT op + two DVE scalar_tensor_tensor ops
per tile). Host transposes the output back.

Sharding: data-parallel over batch across 8 NeuronCores (2048 rows
each); weights replicated. No collectives.
"""

from contextlib import ExitStack

import numpy as np
import ml_dtypes

import concourse.bass as bass
import concourse.tile as tile
from concourse import bacc, mybir
from concourse.bass_utils import run_bass_kernel_spmd

B, D = 16384, 1024
NCORES = 8
BS = B // NCORES   # 2048 batch rows per core
P = 128
TP = 4             # contraction k-pairs (each pair = 256 rows via DoubleRow)
IB = D // P        # 8 output-feature blocks of 128
NBC = BS // 512    # 4 batch chunks of 512

FP8 = mybir.dt.float8e4
FP8_NP = ml_dtypes.float8_e4m3fn  # matches TRN fp8e4 within +-240
BF16_NP = ml_dtypes.bfloat16

_CACHE = {}


def _build_nc():
    nc = bacc.Bacc("TRN2", target_bir_lowering=False, debug=False,
                   num_devices=NCORES)
    # x8: x^T per core in fp8 (PE moving operand), xb: same in bf16 (for
    # the diagonal correction at eviction), tiled [p, tp, 2, b] with
    # j = (2*tp + sub)*128 + p the contraction index.
    x8 = nc.dram_tensor("x8", [P, TP, 2, BS], FP8, kind="ExternalInput").ap()
    xb = nc.dram_tensor("xb", [P, TP, 2, BS], mybir.dt.bfloat16,
                        kind="ExternalInput").ap()
    # W_off (diag zeroed), same contraction tiling: [p, tp, 2, i]
    w8 = nc.dram_tensor("w8", [P, TP, 2, D], FP8, kind="ExternalInput").ap()
    # diag + bias as per-partition columns: dv[p, ib] = d[ib*128+p]
    dv = nc.dram_tensor("dv", [P, IB], mybir.dt.float32,
                        kind="ExternalInput").ap()
    bv = nc.dram_tensor("bv", [P, IB], mybir.dt.float32,
                        kind="ExternalInput").ap()
    # out^T bf16: out[p, ib, b] = result[b, ib*128+p]
    out = nc.dram_tensor("out", [P, IB, BS], mybir.dt.bfloat16,
                         kind="ExternalOutput").ap()

    Act = mybir.ActivationFunctionType
    Alu = mybir.AluOpType
    DR = mybir.MatmulPerfMode.DoubleRow

    with tile.TileContext(nc) as tc, ExitStack() as ctx:
        cpool = ctx.enter_context(tc.tile_pool(name="cpool", bufs=1))
        opool = ctx.enter_context(tc.tile_pool(name="opool", bufs=3))
        tpool = ctx.enter_context(tc.tile_pool(name="tpool", bufs=8))
        ppool = ctx.enter_context(tc.tile_pool(name="ppool", bufs=8,
                                               space="PSUM"))

        # small per-partition vectors first (vector-engine HWDGE ring)
        dv_t = cpool.tile([P, IB], mybir.dt.float32, tag="dv", name="dv_t")
        bv_t = cpool.tile([P, IB], mybir.dt.float32, tag="bv", name="bv_t")
        nc.sync.dma_start(out=dv_t, in_=dv)
        nc.sync.dma_start(out=bv_t, in_=bv)

        # PE pre-warm with fp8 DoubleRow matmuls on a zero tile so the
        # HAM clock-gate ramps while the input DMAs run.
        warm = cpool.tile([P, 2, 512], FP8, tag="warm", name="warm")
        nc.vector.memset(warm, 0.0)
        warm_ps = ppool.tile([P, 512], mybir.dt.float32, tag="ps",
                             name="warm_ps")
        for i in range(6):
            nc.tensor.matmul(warm_ps, lhsT=warm[:, :, :P], rhs=warm,
                             start=(i == 0), stop=(i == 5), perf_mode=DR)

        # weights per k-pair on the scalar ring (first chunk gates PE)
        w_t = []
        for t in range(TP):
            wt = cpool.tile([P, 2, D], FP8, tag=f"w{t}", name=f"w_t{t}")
            nc.scalar.dma_start(out=wt, in_=w8[:, t])
            w_t.append(wt)

        # x^T per k-pair on the sync ring; first pair split by b-chunk so
        # the first matmul's input lands early.
        x_t = []
        for t in range(TP):
            xt = cpool.tile([P, 2, BS], FP8, tag=f"x{t}", name=f"x_t{t}")
            if t == 0:
                for b4 in range(NBC):
                    nc.sync.dma_start(out=xt[:, :, bass.ts(b4, 512)],
                                      in_=x8[:, t, :, bass.ts(b4, 512)])
            else:
                nc.sync.dma_start(out=xt, in_=x8[:, t])
            x_t.append(xt)

        # bf16 x^T on the gpsimd ring (only needed at eviction)
        xb_t = []
        for t in range(TP):
            xbt = cpool.tile([P, 2, BS], mybir.dt.bfloat16, tag=f"xb{t}",
                             name=f"xb_t{t}")
            nc.gpsimd.dma_start(out=xbt, in_=xb[:, t])
            xb_t.append(xbt)

        for ib in range(IB):
            isl = bass.ts(ib, P)
            psums = [ppool.tile([P, 512], mybir.dt.float32, tag="ps",
                                name=f"ps{ib}_{bc}") for bc in range(NBC)]
            for t in range(TP):
                for bc in range(NBC):
                    nc.tensor.matmul(
                        psums[bc],
                        lhsT=w_t[t][:, :, isl],
                        rhs=x_t[t][:, :, bass.ts(bc, 512)],
                        start=(t == 0),
                        stop=(t == TP - 1),
                        perf_mode=DR,
                    )
            # eviction: out^T[i,b] = psum + d[i]*x_bf16[i,b] + bias[i]
            t8, sub = divmod(ib, 2)
            o_t = opool.tile([P, BS], mybir.dt.bfloat16, tag="o",
                             name=f"o_t{ib}")
            for bc in range(NBC):
                bsl = bass.ts(bc, 512)
                tb = tpool.tile([P, 512], mybir.dt.float32, tag="tb",
                                name=f"tb{ib}_{bc}")
                nc.scalar.activation(tb, psums[bc], Act.Identity,
                                     bias=bv_t[:, ib:ib + 1], scale=1.0)
                nc.vector.scalar_tensor_tensor(
                    o_t[:, bsl], in0=xb_t[t8][:, sub, bsl],
                    scalar=dv_t[:, ib:ib + 1],
                    in1=tb, op0=Alu.mult, op1=Alu.add)
                if ib == IB - 1:
                    # pipeline the kernel tail: store per 512-chunk
                    nc.scalar.dma_start(out=out[:, ib, bsl],
                                        in_=o_t[:, bsl])
            if ib < IB - 1:
                nc.scalar.dma_start(out=out[:, ib], in_=o_t)

    nc.compile()
    return nc


def _get_nc():
    if "nc" not in _CACHE:
        _CACHE["nc"] = _build_nc()
    return _CACHE["nc"]


def _prep_inputs(x, lin_W, lin_b, W1, b1, W2):
    """Host-side prep: fold the basis chain, split W into off-diag + diag,
    quantize to fp8 (value + residual), and lay out transposed per core."""
    x = np.asarray(x, dtype=np.float32)
    lin_W = np.asarray(lin_W, dtype=np.float32)
    a = np.sum(np.asarray(W1, np.float32) * np.asarray(W2, np.float32),
               axis=1)
    c = np.sum(np.asarray(b1, np.float32) * np.asarray(W2, np.float32),
               axis=1)
    W_eff = np.ascontiguousarray(lin_W.T)
    idx = np.arange(D)
    W_eff[idx, idx] += a
    d = W_eff[idx, idx].copy()
    W_off = W_eff
    W_off[idx, idx] = 0.0
    bias = (np.asarray(lin_b, np.float32) + c).astype(np.float32)

    x8 = x.astype(FP8_NP)
    xb = x.astype(BF16_NP)
    w8 = W_off.astype(FP8_NP)

    # w8 dram layout [p, tp, 2, i]: j = (2*tp+sub)*128 + p
    w8_dev = np.ascontiguousarray(
        w8.reshape(TP, 2, P, D).transpose(2, 0, 1, 3))
    dv_dev = np.ascontiguousarray(d.reshape(IB, P).T)
    bv_dev = np.ascontiguousarray(bias.reshape(IB, P).T)

    def xpose(arr):  # [NCORES*BS, D] fp8 -> per-core [p, tp, 2, b]
        t = arr.reshape(NCORES, BS, TP, 2, P)
        return np.ascontiguousarray(t.transpose(0, 4, 2, 3, 1))

    x8_dev = xpose(x8)
    xb_dev = xpose(xb)

    return [
        {"x8": x8_dev[i], "xb": xb_dev[i], "w8": w8_dev,
         "dv": dv_dev, "bv": bv_dev}
        for i in range(NCORES)
    ]


def kernel(x, lin_W, lin_b, W1, b1, W2):
    in_maps = _prep_inputs(x, lin_W, lin_b, W1, b1, W2)
    nc = _get_nc()
    res = run_bass_kernel_spmd(nc, in_maps, core_ids=list(range(NCORES)))
    # out^T [p, ib, b] per core -> [b_global, ib*128+p]
    o = np.stack([r["out"] for r in res.results])  # [cores, P, IB, BS] bf16
    o = o.astype(np.float32).transpose(0, 3, 2, 1).reshape(B, D)
    return np.ascontiguousarray(o)
